# revision 24
# baseline (speedup 1.0000x reference)
"""Trainium2 Bass kernel for nn_AttentionBlock (B=4, H=W=64, C=64, GroupNorm(8) +
full spatial self-attention), distributed over 8 NeuronCores.

Sharding: core i handles batch b=i//2 and query-half h=i%2 (2048 of the 4096
spatial positions). Each core computes the full GroupNorm and K/V for its
image (cheap) and attention only for its query half. No collectives.

Device layout: channel-on-partition ("xT") layout, with the two position
halves of an image packed onto partitions [(half, channel)] -> 128 partitions.
Scores are computed transposed (kv positions on partitions) so that
- the softmax denominator is a free by-product of the attn@V matmul
  (all-ones column appended to V), and
- the attn@V contraction needs no transposes at all.
exp() runs on ScalarE reading PSUM and writing SBUF directly; everything is
software-pipelined per 3-chunk batch: scores (PE) -> exp (ACT) -> attn@V (PE)
with a lookahead so both engines stay saturated.

PE matmuls run in bf16 (scores get exact K=128 via zero-padded query
operands, which also keeps the PE activity monitor from downclocking);
GroupNorm statistics, PSUM accumulation and the residual path stay fp32.
"""

import sys

sys.path.insert(0, "/opt/trn_rl_repo")

import numpy as np

import concourse.bacc as bacc
import concourse.tile as tile
from concourse import mybir

B, H, W, C = 4, 64, 64, 64
HW = H * W  # 4096
HALF = HW // 2  # 2048
EPS = 1e-5
SCALE = C ** -0.5  # folded into exp()

F32 = mybir.dt.float32
MDT = mybir.dt.bfloat16  # PE matmul operand dtype

EXP_BATCH = 3  # kv chunks (PSUM banks) per exp() call
NBATCH = 11  # ceil(32 / EXP_BATCH)
BATCHES = [list(range(t, min(t + EXP_BATCH, 32))) for t in range(0, 32, EXP_BATCH)]


def build_nc():
    nc = bacc.Bacc("TRN2", debug=False, num_devices=8)

    # ---- DRAM I/O ----
    xp_d = nc.dram_tensor("xp", [128, HALF], F32, kind="ExternalInput")
    wq_d = nc.dram_tensor("wq", [64, 128], MDT, kind="ExternalInput")
    wk_d = nc.dram_tensor("wk", [128, 128], MDT, kind="ExternalInput")
    wv_d = nc.dram_tensor("wv", [128, 128], MDT, kind="ExternalInput")
    wo_d = nc.dram_tensor("wo", [64, 64], MDT, kind="ExternalInput")
    bq_d = nc.dram_tensor("bq", [1, 128], MDT, kind="ExternalInput")
    bk_d = nc.dram_tensor("bk", [1, 128], MDT, kind="ExternalInput")
    bv_d = nc.dram_tensor("bv", [1, 128], MDT, kind="ExternalInput")
    bo_d = nc.dram_tensor("bo", [64, 1], F32, kind="ExternalInput")
    gam_d = nc.dram_tensor("gam", [128, 1], F32, kind="ExternalInput")
    bet_d = nc.dram_tensor("bet", [128, 1], F32, kind="ExternalInput")
    comb_d = nc.dram_tensor("comb", [128, 128], F32, kind="ExternalInput")
    out_d = nc.dram_tensor("out", [64, HALF], F32, kind="ExternalOutput")

    with tile.TileContext(nc) as tc, \
         tc.tile_pool(name="singles", bufs=1) as singles, \
         tc.tile_pool(name="stats", bufs=1) as stats, \
         tc.tile_pool(name="sc_ps", bufs=2, space="PSUM") as sc_ps, \
         tc.tile_pool(name="pacc_ps", bufs=2, space="PSUM") as pacc_ps, \
         tc.tile_pool(name="fin_ps", bufs=2, space="PSUM") as fin_ps, \
         tc.tile_pool(name="work", bufs=2) as work:

        # ---- input DMAs: x on the sync queue, weights on gpsimd ----
        x_sb = singles.tile([128, HALF], F32)
        for r in range(4):
            nc.sync.dma_start(
                x_sb[:, 512 * r : 512 * r + 512],
                xp_d.ap()[:, 512 * r : 512 * r + 512],
            )
        wq_sb = singles.tile([64, 128], MDT)
        nc.gpsimd.dma_start(wq_sb[:], wq_d.ap())
        wk_sb = singles.tile([128, 128], MDT)
        nc.gpsimd.dma_start(wk_sb[:], wk_d.ap())
        bq_sb = singles.tile([1, 128], MDT)
        nc.gpsimd.dma_start(bq_sb[:], bq_d.ap())
        bk_sb = singles.tile([1, 128], MDT)
        nc.gpsimd.dma_start(bk_sb[:], bk_d.ap())
        gam_sb = singles.tile([128, 1], F32)
        nc.gpsimd.dma_start(gam_sb[:], gam_d.ap())
        bet_sb = singles.tile([128, 1], F32)
        nc.gpsimd.dma_start(bet_sb[:], bet_d.ap())
        comb_sb = singles.tile([128, 128], F32)
        nc.gpsimd.dma_start(comb_sb[:], comb_d.ap())
        wv_sb = singles.tile([128, 128], MDT)
        nc.gpsimd.dma_start(wv_sb[:], wv_d.ap())
        wo_sb = singles.tile([64, 64], MDT)
        nc.gpsimd.dma_start(wo_sb[:], wo_d.ap())
        bv_sb = singles.tile([1, 128], MDT)
        nc.gpsimd.dma_start(bv_sb[:], bv_d.ap())
        bo_sb = singles.tile([64, 1], F32)
        nc.gpsimd.dma_start(bo_sb[:], bo_d.ap())

        # ---- big SBUF tensors ----
        xn_r = singles.tile([128, HALF], MDT)
        q_dup = singles.tile([128, HALF], MDT)
        kt_sb = singles.tile([128, HALF], MDT)
        v_all = singles.tile([128, 65 * 32], MDT)
        attnexp = singles.tile([128, 512 * 32], MDT)
        out_sb = singles.tile([64, HALF], F32)
        ones_sb = singles.tile([128, 512], MDT)

        ones32 = singles.tile([65, 64], F32)
        # constants via gpsimd (DVE stays free for the stats chain)
        nc.gpsimd.memset(ones32[:], 1.0)
        nc.gpsimd.memset(ones_sb[:], 1.0)
        v3 = v_all[:].rearrange("p (t e) -> p t e", e=65)
        nc.gpsimd.memset(v3[:, :, 64:65], 1.0)

        # pre-warm the exp ACT table set under the DMA shadow
        scr = stats.tile([128, 1], F32)
        nc.vector.memset(scr[:], 1.0)
        nc.scalar.activation(scr[:], scr[:], mybir.ActivationFunctionType.Exp)

        # ---- GroupNorm stats: bn per partition per 512-slice, then a
        # block-diagonal averaging matmul combines across channels ----
        st6 = stats.tile([128, 4, 6], F32)
        mv4 = stats.tile([128, 4, 2], F32)
        for r in range(4):
            nc.vector.bn_stats(st6[:, r, :], x_sb[:, 512 * r : 512 * r + 512])
            nc.vector.bn_aggr(mv4[:, r, :], st6[:, r, :])
        smat = stats.tile([128, 8], F32)  # cols 0-3 mean, 4-7 E[x^2]
        nc.vector.tensor_copy(smat[:, 0:4], mv4[:, :, 0])
        nc.vector.tensor_mul(smat[:, 4:8], mv4[:, :, 0], mv4[:, :, 0])
        nc.vector.tensor_add(smat[:, 4:8], smat[:, 4:8], mv4[:, :, 1])

        cps = fin_ps.tile([128, 8], F32, tag="fin")
        nc.tensor.matmul(cps[:], comb_sb[:], smat[:], start=True, stop=True)
        gstat = stats.tile([128, 8], F32)  # 0-3 mean_g, 4-7 E2_g
        nc.vector.tensor_copy(gstat[:], cps[:])

        # var+eps, then rstd = rsqrt via bit-trick seed + Newton steps (DVE)
        ve = stats.tile([128, 4], F32)
        nc.vector.tensor_mul(ve[:], gstat[:, 0:4], gstat[:, 0:4])
        nc.vector.tensor_scalar(
            out=ve[:], in0=ve[:], scalar1=-1.0, scalar2=EPS,
            op0=mybir.AluOpType.mult, op1=mybir.AluOpType.add,
        )
        nc.vector.tensor_add(ve[:], ve[:], gstat[:, 4:8])
        yi = stats.tile([128, 4], mybir.dt.int32)
        nc.vector.tensor_scalar(
            out=yi[:], in0=ve[:].bitcast(mybir.dt.int32), scalar1=1,
            scalar2=None, op0=mybir.AluOpType.logical_shift_right,
        )
        nc.vector.tensor_scalar(
            out=yi[:], in0=yi[:], scalar1=-1, scalar2=0x5F3759DF,
            op0=mybir.AluOpType.mult, op1=mybir.AluOpType.add,
        )
        rstd = stats.tile([128, 4], F32)
        nc.vector.tensor_copy(rstd[:], yi[:].bitcast(F32))
        vh = stats.tile([128, 4], F32)
        nc.vector.tensor_scalar_mul(vh[:], ve[:], -0.5)
        t_nw = stats.tile([128, 4], F32)
        for _ in range(2):
            nc.vector.tensor_mul(t_nw[:], rstd[:], rstd[:])
            nc.vector.tensor_mul(t_nw[:], t_nw[:], vh[:])
            nc.vector.tensor_scalar(
                out=t_nw[:], in0=t_nw[:], scalar1=1.0, scalar2=1.5,
                op0=mybir.AluOpType.mult, op1=mybir.AluOpType.add,
            )
            nc.vector.tensor_mul(rstd[:], rstd[:], t_nw[:])

        gsc = stats.tile([128, 4], F32)
        nc.vector.tensor_scalar_mul(gsc[:], rstd[:], gam_sb[:])
        gbias = stats.tile([128, 4], F32)
        nc.vector.tensor_mul(gbias[:], gstat[:, 0:4], gsc[:])
        nc.vector.tensor_scalar(
            out=gbias[:], in0=gbias[:], scalar1=-1.0, scalar2=bet_sb[:],
            op0=mybir.AluOpType.mult, op1=mybir.AluOpType.add,
        )
        # xn = x * gsc + gbias: bf16 copy for the matmuls via ScalarE (runs
        # parallel to the DVE fp32 pass used by the residual path)
        for r in range(4):
            sl = slice(512 * r, 512 * r + 512)
            nc.scalar.activation(
                xn_r[:, sl], x_sb[:, sl],
                mybir.ActivationFunctionType.Identity,
                bias=gbias[:, r : r + 1], scale=gsc[:, r : r + 1],
            )
            nc.vector.tensor_scalar(
                out=x_sb[:, sl], in0=x_sb[:, sl],
                scalar1=gsc[:, r : r + 1], scalar2=gbias[:, r : r + 1],
                op0=mybir.AluOpType.mult, op1=mybir.AluOpType.add,
            )

        # ---- emission helpers ----
        def emit_qk_slice(t):
            # q^T duplicated on both partition halves (lhsT = [Wq | Wq]);
            # zero-padded into q0/q1 so scores run K=128. k^T packed by half
            # (lhsT = blockdiag(Wk, Wk)). Copies split across ACT and DVE.
            sl = slice(512 * t, 512 * t + 512)
            ps = fin_ps.tile([128, 512], F32, tag="fin", name=f"qps{t}")
            nc.tensor.matmul(ps[:], bq_sb[:], ones_sb[0:1, :], start=True,
                             stop=False)
            nc.tensor.matmul(ps[:], wq_sb[:], xn_r[0:64, sl], start=False,
                             stop=True)
            nc.vector.tensor_copy(q_dup[:, sl], ps[:])
            ps2 = fin_ps.tile([128, 512], F32, tag="fin", name=f"kps{t}")
            nc.tensor.matmul(ps2[:], bk_sb[:], ones_sb[0:1, :], start=True,
                             stop=False)
            nc.tensor.matmul(ps2[:], wk_sb[:], xn_r[:, sl], start=False,
                             stop=True)
            nc.vector.tensor_copy(kt_sb[:, sl], ps2[:])

        def emit_v_pair(u):
            # v position-major, two 128-position chunks per matmul
            sl = slice(128 * u, 128 * u + 128)
            ps = fin_ps.tile([128, 128], F32, tag="fin", name=f"vps{u}")
            nc.tensor.matmul(ps[:], ones_sb[0:1, 0:128], bv_sb[:], start=True,
                             stop=False)
            nc.tensor.matmul(ps[:], xn_r[:, sl], wv_sb[:], start=False,
                             stop=True)
            nc.vector.tensor_copy(v_all[:, 65 * u : 65 * u + 64], ps[:, 0:64])
            nc.vector.tensor_copy(
                v_all[:, 65 * (u + 16) : 65 * (u + 16) + 64], ps[:, 64:128]
            )

        def aoff(t):
            # attnexp is pair-major: chunk c at 1024c, chunk c+16 at 1024c+512
            return 1024 * t if t < 16 else 1024 * (t - 16) + 512

        def emit_scores_batch(n, bi):
            # kv chunk pair (bi, bi+16): two K=64 matmuls issued back-to-back
            # into row groups 0 and 64 run concurrently on the PE array
            qsl = slice(512 * n, 512 * n + 512)
            ps = sc_ps.tile([128, 1024], F32, tag="sc", name=f"scps{n}_{bi}")
            ksl = slice(128 * bi, 128 * bi + 128)
            nc.tensor.matmul(ps[:, 0:512], kt_sb[0:64, ksl],
                             q_dup[0:64, qsl], start=True, stop=True)
            nc.tensor.matmul(ps[:, 512:1024], kt_sb[64:128, ksl],
                             q_dup[64:128, qsl], start=True, stop=True)
            nc.scalar.activation(
                attnexp[:, 1024 * bi : 1024 * bi + 1024], ps[:],
                mybir.ActivationFunctionType.Exp, scale=SCALE,
            )

        paccs = {}

        def emit_proj_batch(n, bi):
            # kv chunk pair (bi, bi+16) — matches exp production order
            if n not in paccs:
                paccs[n] = pacc_ps.tile([65, 512], F32, tag="pacc",
                                        name=f"pacc{n}")
            pacc = paccs[n]
            for t in (bi, bi + 16):
                nc.tensor.matmul(
                    pacc[:], v_all[:, 65 * t : 65 * t + 65],
                    attnexp[:, aoff(t) : aoff(t) + 512],
                    start=(t == 0), stop=(t == 31),
                )

        def emit_finish_a(n):
            # free the PSUM accumulator ASAP: unnormalized proj rows (bf16,
            # feeds the out-projection) + raw denominator row
            pacc = paccs[n]
            projn_u = work.tile([64, 512], MDT, tag="projn", name=f"pn{n}")
            nc.vector.tensor_copy(projn_u[:], pacc[0:64, :])
            dn_sb = work.tile([65, 512], F32, tag="dn", name=f"dn{n}")
            nc.vector.tensor_copy(dn_sb[64:65, :], pacc[64:65, :])
            return projn_u, dn_sb

        def emit_finish_b(n, projn_u, dn_sb):
            # PE: broadcast raw denom + out-projection (nothing here waits on
            # a reciprocal). DVE then normalizes and applies bias + residual.
            qsl = slice(512 * n, 512 * n + 512)
            bc_ps = fin_ps.tile([64, 512], F32, tag="fin", name=f"bc{n}")
            nc.tensor.matmul(bc_ps[:], ones32[64:65, :], dn_sb[64:65, :],
                             start=True, stop=True)
            fps = fin_ps.tile([64, 512], F32, tag="fin", name=f"fps{n}")
            nc.tensor.matmul(fps[:], wo_sb[:], projn_u[:], start=True, stop=True)
            bc_sb = work.tile([64, 512], F32, tag="bc", name=f"bcs{n}")
            nc.vector.tensor_copy(bc_sb[:], bc_ps[:])
            with nc.allow_low_precision(reason="softmax denom reciprocal"):
                nc.vector.reciprocal(bc_sb[:], bc_sb[:])
            xb = work.tile([64, 512], F32, tag="xb", name=f"xb{n}")
            nc.vector.tensor_scalar_add(xb[:], x_sb[0:64, qsl], bo_sb[:])
            mn = work.tile([64, 512], F32, tag="mn", name=f"mn{n}")
            nc.vector.tensor_mul(mn[:], fps[:], bc_sb[:])
            nc.vector.tensor_add(out_sb[:, qsl], mn[:], xb[:])
            nc.sync.dma_start(out_d.ap()[:, qsl], out_sb[:, qsl])

        # ---- software-pipelined attention: 4 tiles x 16 pair-slots.
        # Per slot: scores-pair + exp(bi), attn@V of chunks 2(bi-2)..; the
        # previous tile's two spill batches and finish ride slots 0-1.
        # Tile 0's slots also carry the q/k slices and the v pairs. ----
        emit_qk_slice(0)
        V_SCHED = {0: [0, 1], 1: [2, 3], 2: [4], 3: [5], 4: [6, 7], 5: [8],
                   6: [9], 7: [10, 11], 8: [12], 9: [13], 10: [14], 11: [15]}
        QK_SCHED = {1: 1, 4: 2, 7: 3}

        pend = {}

        def emit_head(n, bi):
            if bi == 0:
                emit_proj_batch(n - 1, 14)
                emit_proj_batch(n - 1, 15)
            elif bi == 1:
                pend[n - 1] = emit_finish_a(n - 1)
            elif bi == 2:
                emit_finish_b(n - 1, *pend.pop(n - 1))

        for n in range(4):
            for bi in range(16):
                emit_scores_batch(n, bi)
                if n == 0:
                    if bi in QK_SCHED:
                        emit_qk_slice(QK_SCHED[bi])
                    for u in V_SCHED.get(bi, []):
                        emit_v_pair(u)
                else:
                    emit_head(n, bi)
                if bi >= 2:
                    emit_proj_batch(n, bi - 2)
        for bi in range(7):
            emit_head(4, bi)

    nc.compile()
    return nc


def host_prep(x, gamma, beta, Wq, bq, Wk, bk, Wv, bv, Wo, bo):
    """Build the 8 per-core input dicts."""
    f32 = lambda a: np.ascontiguousarray(np.asarray(a, np.float32))
    x = f32(x)
    gamma, beta = f32(gamma), f32(beta)
    Wq, Wk, Wv, Wo = f32(Wq), f32(Wk), f32(Wv), f32(Wo)
    bq, bk, bv, bo = f32(bq), f32(bk), f32(bv), f32(bo)

    wq_dup = np.ascontiguousarray(np.concatenate([Wq, Wq], axis=1))
    z = np.zeros((64, 64), np.float32)
    wk_blk = np.ascontiguousarray(np.block([[Wk, z], [z, Wk]]))
    wv_blk = np.ascontiguousarray(np.block([[Wv, z], [z, Wv]]))
    comb = np.zeros((128, 128), np.float32)
    comb[:64, :64] = 1.0 / 64.0
    comb[64:, 64:] = 1.0 / 64.0
    mdt_np = mybir.dt.np(MDT)
    m = lambda a: np.ascontiguousarray(a).astype(mdt_np)
    shared = {
        "wq": m(wq_dup), "wk": m(wk_blk), "wv": m(wv_blk), "wo": m(Wo),
        "bq": m(np.tile(bq, 2)[None]),
        "bk": m(np.tile(bk, 2)[None]),
        "bv": m(np.tile(bv, 2)[None]),
        "bo": np.ascontiguousarray(bo[:, None]),
        "gam": np.ascontiguousarray(np.tile(gamma, 2)[:, None]),
        "bet": np.ascontiguousarray(np.tile(beta, 2)[:, None]),
        "comb": comb,
    }
    in_maps = []
    for core in range(8):
        b, h = core // 2, core % 2
        xT = x[b].reshape(HW, C).T  # [64, 4096]
        halves = xT.reshape(C, 2, HALF)[:, [h, 1 - h], :]
        xp = np.ascontiguousarray(halves.transpose(1, 0, 2).reshape(128, HALF))
        in_maps.append({"xp": xp, **shared})
    return in_maps


def assemble(results, dtype):
    out = np.empty((B, HW, C), np.float32)
    for core in range(8):
        b, h = core // 2, core % 2
        out[b, HALF * h : HALF * h + HALF] = results[core]["out"].T
    return out.reshape(B, H, W, C).astype(dtype, copy=False)


_NC_CACHE = []


def kernel(x, gamma, beta, Wq, bq, Wk, bk, Wv, bv, Wo, bo):
    from concourse.bass_utils import run_bass_kernel_spmd

    if not _NC_CACHE:
        _NC_CACHE.append(build_nc())
    nc = _NC_CACHE[0]
    in_maps = host_prep(x, gamma, beta, Wq, bq, Wk, bk, Wv, bv, Wo, bo)
    res = run_bass_kernel_spmd(nc, in_maps, core_ids=list(range(8)))
    return assemble(res.results, np.asarray(x).dtype)


if __name__ == "__main__":
    rng = np.random.default_rng(0)
    inputs = {
        "x": rng.standard_normal((B, H, W, C)).astype(np.float32),
        "gamma": np.ones(C, np.float32), "beta": np.zeros(C, np.float32),
        "Wq": (rng.standard_normal((C, C)) / 8).astype(np.float32),
        "bq": np.zeros(C, np.float32),
        "Wk": (rng.standard_normal((C, C)) / 8).astype(np.float32),
        "bk": np.zeros(C, np.float32),
        "Wv": (rng.standard_normal((C, C)) / 8).astype(np.float32),
        "bv": np.zeros(C, np.float32),
        "Wo": (rng.standard_normal((C, C)) / 8).astype(np.float32),
        "bo": np.zeros(C, np.float32),
    }
    out = kernel(**inputs)
    print("kernel ran, out shape", out.shape, out.dtype)


# revision 25
# speedup vs baseline: 1.1062x; 1.1062x over previous
"""Trainium2 Bass kernel for nn_AttentionBlock (B=4, H=W=64, C=64, GroupNorm(8) +
full spatial self-attention), distributed over 8 NeuronCores.

Sharding: core i handles batch b=i//2 and query-half h=i%2 (2048 of the 4096
spatial positions). Each core computes the full GroupNorm and K/V for its
image (cheap) and attention only for its query half. No collectives.

Device layout: channel-on-partition ("xT") layout, with the two position
halves of an image packed onto partitions [(half, channel)] -> 128 partitions.
Scores are computed transposed (kv positions on partitions) so that
- the softmax denominator is a free by-product of the attn@V matmul
  (all-ones column appended to V), and
- the attn@V contraction needs no transposes at all.
exp() runs on ScalarE reading PSUM and writing SBUF directly; everything is
software-pipelined per 3-chunk batch: scores (PE) -> exp (ACT) -> attn@V (PE)
with a lookahead so both engines stay saturated.

PE matmuls run in bf16 (scores get exact K=128 via zero-padded query
operands, which also keeps the PE activity monitor from downclocking);
GroupNorm statistics, PSUM accumulation and the residual path stay fp32.
"""

import sys

sys.path.insert(0, "/opt/trn_rl_repo")

import numpy as np

import concourse.bacc as bacc
import concourse.tile as tile
from concourse import mybir

B, H, W, C = 4, 64, 64, 64
HW = H * W  # 4096
HALF = HW // 2  # 2048
EPS = 1e-5
SCALE = C ** -0.5  # folded into exp()

F32 = mybir.dt.float32
MDT = mybir.dt.bfloat16  # PE matmul operand dtype

EXP_BATCH = 3  # kv chunks (PSUM banks) per exp() call
NBATCH = 11  # ceil(32 / EXP_BATCH)
BATCHES = [list(range(t, min(t + EXP_BATCH, 32))) for t in range(0, 32, EXP_BATCH)]


def build_nc():
    nc = bacc.Bacc("TRN2", debug=False, num_devices=8)

    # ---- DRAM I/O ----
    xp_d = nc.dram_tensor("xp", [128, HALF], F32, kind="ExternalInput")
    wq_d = nc.dram_tensor("wq", [64, 128], MDT, kind="ExternalInput")
    wk_d = nc.dram_tensor("wk", [128, 128], MDT, kind="ExternalInput")
    wv_d = nc.dram_tensor("wv", [128, 128], MDT, kind="ExternalInput")
    wo_d = nc.dram_tensor("wo", [64, 64], MDT, kind="ExternalInput")
    bq_d = nc.dram_tensor("bq", [1, 128], MDT, kind="ExternalInput")
    bk_d = nc.dram_tensor("bk", [1, 128], MDT, kind="ExternalInput")
    bv_d = nc.dram_tensor("bv", [1, 128], MDT, kind="ExternalInput")
    bo_d = nc.dram_tensor("bo", [64, 1], F32, kind="ExternalInput")
    gam_d = nc.dram_tensor("gam", [128, 1], F32, kind="ExternalInput")
    bet_d = nc.dram_tensor("bet", [128, 1], F32, kind="ExternalInput")
    comb_d = nc.dram_tensor("comb", [128, 128], F32, kind="ExternalInput")
    out_d = nc.dram_tensor("out", [64, HALF], F32, kind="ExternalOutput")

    with tile.TileContext(nc) as tc, \
         tc.tile_pool(name="singles", bufs=1) as singles, \
         tc.tile_pool(name="stats", bufs=1) as stats, \
         tc.tile_pool(name="sc_ps", bufs=2, space="PSUM") as sc_ps, \
         tc.tile_pool(name="pacc_ps", bufs=2, space="PSUM") as pacc_ps, \
         tc.tile_pool(name="fin_ps", bufs=2, space="PSUM") as fin_ps, \
         tc.tile_pool(name="work", bufs=2) as work:

        # ---- input DMAs: x on the sync queue, weights on gpsimd ----
        x_sb = singles.tile([128, HALF], F32)
        for r in range(4):
            nc.sync.dma_start(
                x_sb[:, 512 * r : 512 * r + 512],
                xp_d.ap()[:, 512 * r : 512 * r + 512],
            )
        wq_sb = singles.tile([64, 128], MDT)
        nc.gpsimd.dma_start(wq_sb[:], wq_d.ap())
        wk_sb = singles.tile([128, 128], MDT)
        nc.gpsimd.dma_start(wk_sb[:], wk_d.ap())
        bq_sb = singles.tile([1, 128], MDT)
        nc.gpsimd.dma_start(bq_sb[:], bq_d.ap())
        bk_sb = singles.tile([1, 128], MDT)
        nc.gpsimd.dma_start(bk_sb[:], bk_d.ap())
        gam_sb = singles.tile([128, 1], F32)
        nc.gpsimd.dma_start(gam_sb[:], gam_d.ap())
        bet_sb = singles.tile([128, 1], F32)
        nc.gpsimd.dma_start(bet_sb[:], bet_d.ap())
        comb_sb = singles.tile([128, 128], F32)
        nc.gpsimd.dma_start(comb_sb[:], comb_d.ap())
        wv_sb = singles.tile([128, 128], MDT)
        nc.gpsimd.dma_start(wv_sb[:], wv_d.ap())
        wo_sb = singles.tile([64, 64], MDT)
        nc.gpsimd.dma_start(wo_sb[:], wo_d.ap())
        bv_sb = singles.tile([1, 128], MDT)
        nc.gpsimd.dma_start(bv_sb[:], bv_d.ap())
        bo_sb = singles.tile([64, 1], F32)
        nc.gpsimd.dma_start(bo_sb[:], bo_d.ap())

        # ---- big SBUF tensors ----
        xn_r = singles.tile([128, HALF], MDT)
        q_dup = singles.tile([128, HALF], MDT)
        kt_sb = singles.tile([128, HALF], MDT)
        v_all = singles.tile([128, 65 * 32], MDT)
        attnexp = singles.tile([128, 512 * 32], MDT)
        out_sb = singles.tile([64, HALF], F32)
        ones_sb = singles.tile([128, 512], MDT)

        ones32 = singles.tile([65, 64], F32)
        # constants via gpsimd (DVE stays free for the stats chain)
        nc.gpsimd.memset(ones32[:], 1.0)
        nc.gpsimd.memset(ones_sb[:], 1.0)
        v3 = v_all[:].rearrange("p (t e) -> p t e", e=65)
        nc.gpsimd.memset(v3[:, :, 64:65], 1.0)

        # pre-warm the exp ACT table set under the DMA shadow
        scr = stats.tile([128, 1], F32)
        nc.vector.memset(scr[:], 1.0)
        nc.scalar.activation(scr[:], scr[:], mybir.ActivationFunctionType.Exp)

        # ---- GroupNorm stats: bn per partition per 512-slice, then a
        # block-diagonal averaging matmul combines across channels ----
        st6 = stats.tile([128, 4, 6], F32)
        mv4 = stats.tile([128, 4, 2], F32)
        for r in range(4):
            nc.vector.bn_stats(st6[:, r, :], x_sb[:, 512 * r : 512 * r + 512])
            nc.vector.bn_aggr(mv4[:, r, :], st6[:, r, :])
        smat = stats.tile([128, 8], F32)  # cols 0-3 mean, 4-7 E[x^2]
        nc.vector.tensor_copy(smat[:, 0:4], mv4[:, :, 0])
        nc.vector.tensor_mul(smat[:, 4:8], mv4[:, :, 0], mv4[:, :, 0])
        nc.vector.tensor_add(smat[:, 4:8], smat[:, 4:8], mv4[:, :, 1])

        cps = fin_ps.tile([128, 8], F32, tag="fin")
        nc.tensor.matmul(cps[:], comb_sb[:], smat[:], start=True, stop=True)
        gstat = stats.tile([128, 8], F32)  # 0-3 mean_g, 4-7 E2_g
        nc.vector.tensor_copy(gstat[:], cps[:])

        # var+eps, then rstd = rsqrt via bit-trick seed + Newton steps (DVE)
        ve = stats.tile([128, 4], F32)
        nc.vector.tensor_mul(ve[:], gstat[:, 0:4], gstat[:, 0:4])
        nc.vector.tensor_scalar(
            out=ve[:], in0=ve[:], scalar1=-1.0, scalar2=EPS,
            op0=mybir.AluOpType.mult, op1=mybir.AluOpType.add,
        )
        nc.vector.tensor_add(ve[:], ve[:], gstat[:, 4:8])
        yi = stats.tile([128, 4], mybir.dt.int32)
        nc.vector.tensor_scalar(
            out=yi[:], in0=ve[:].bitcast(mybir.dt.int32), scalar1=1,
            scalar2=None, op0=mybir.AluOpType.logical_shift_right,
        )
        nc.vector.tensor_scalar(
            out=yi[:], in0=yi[:], scalar1=-1, scalar2=0x5F3759DF,
            op0=mybir.AluOpType.mult, op1=mybir.AluOpType.add,
        )
        rstd = stats.tile([128, 4], F32)
        nc.vector.tensor_copy(rstd[:], yi[:].bitcast(F32))
        vh = stats.tile([128, 4], F32)
        nc.vector.tensor_scalar_mul(vh[:], ve[:], -0.5)
        t_nw = stats.tile([128, 4], F32)
        for _ in range(2):
            nc.vector.tensor_mul(t_nw[:], rstd[:], rstd[:])
            nc.vector.tensor_mul(t_nw[:], t_nw[:], vh[:])
            nc.vector.tensor_scalar(
                out=t_nw[:], in0=t_nw[:], scalar1=1.0, scalar2=1.5,
                op0=mybir.AluOpType.mult, op1=mybir.AluOpType.add,
            )
            nc.vector.tensor_mul(rstd[:], rstd[:], t_nw[:])

        gsc = stats.tile([128, 4], F32)
        nc.vector.tensor_scalar_mul(gsc[:], rstd[:], gam_sb[:])
        gbias = stats.tile([128, 4], F32)
        nc.vector.tensor_mul(gbias[:], gstat[:, 0:4], gsc[:])
        nc.vector.tensor_scalar(
            out=gbias[:], in0=gbias[:], scalar1=-1.0, scalar2=bet_sb[:],
            op0=mybir.AluOpType.mult, op1=mybir.AluOpType.add,
        )
        # xn = x * gsc + gbias: bf16 copy for the matmuls via ScalarE (runs
        # parallel to the DVE fp32 pass used by the residual path)
        for r in range(4):
            sl = slice(512 * r, 512 * r + 512)
            nc.scalar.activation(
                xn_r[:, sl], x_sb[:, sl],
                mybir.ActivationFunctionType.Identity,
                bias=gbias[:, r : r + 1], scale=gsc[:, r : r + 1],
            )
            nc.vector.tensor_scalar(
                out=x_sb[:, sl], in0=x_sb[:, sl],
                scalar1=gsc[:, r : r + 1], scalar2=gbias[:, r : r + 1],
                op0=mybir.AluOpType.mult, op1=mybir.AluOpType.add,
            )

        # ---- emission helpers ----
        def emit_qk_slice(t):
            # q^T duplicated on both partition halves (lhsT = [Wq | Wq]);
            # zero-padded into q0/q1 so scores run K=128. k^T packed by half
            # (lhsT = blockdiag(Wk, Wk)). Copies split across ACT and DVE.
            sl = slice(512 * t, 512 * t + 512)
            ps = fin_ps.tile([128, 512], F32, tag="fin", name=f"qps{t}")
            nc.tensor.matmul(ps[:], bq_sb[:], ones_sb[0:1, :], start=True,
                             stop=False)
            nc.tensor.matmul(ps[:], wq_sb[:], xn_r[0:64, sl], start=False,
                             stop=True)
            nc.scalar.copy(q_dup[:, sl], ps[:])
            ps2 = fin_ps.tile([128, 512], F32, tag="fin", name=f"kps{t}")
            nc.tensor.matmul(ps2[:], bk_sb[:], ones_sb[0:1, :], start=True,
                             stop=False)
            nc.tensor.matmul(ps2[:], wk_sb[:], xn_r[:, sl], start=False,
                             stop=True)
            nc.vector.tensor_copy(kt_sb[:, sl], ps2[:])

        def emit_v_pair(u):
            # v position-major, two 128-position chunks per matmul
            sl = slice(128 * u, 128 * u + 128)
            ps = fin_ps.tile([128, 128], F32, tag="fin", name=f"vps{u}")
            nc.tensor.matmul(ps[:], ones_sb[0:1, 0:128], bv_sb[:], start=True,
                             stop=False)
            nc.tensor.matmul(ps[:], xn_r[:, sl], wv_sb[:], start=False,
                             stop=True)
            nc.vector.tensor_copy(v_all[:, 65 * u : 65 * u + 64], ps[:, 0:64])
            nc.vector.tensor_copy(
                v_all[:, 65 * (u + 16) : 65 * (u + 16) + 64], ps[:, 64:128]
            )

        def aoff(t):
            # attnexp is pair-major: chunk c at 1024c, chunk c+16 at 1024c+512
            return 1024 * t if t < 16 else 1024 * (t - 16) + 512

        def emit_scores_batch(n, bi):
            # kv chunk pair (bi, bi+16): two K=64 matmuls issued back-to-back
            # into row groups 0 and 64 run concurrently on the PE array
            qsl = slice(512 * n, 512 * n + 512)
            ps = sc_ps.tile([128, 1024], F32, tag="sc", name=f"scps{n}_{bi}")
            ksl = slice(128 * bi, 128 * bi + 128)
            nc.tensor.matmul(ps[:, 0:512], kt_sb[0:64, ksl],
                             q_dup[0:64, qsl], start=True, stop=True)
            nc.tensor.matmul(ps[:, 512:1024], kt_sb[64:128, ksl],
                             q_dup[64:128, qsl], start=True, stop=True)
            nc.scalar.activation(
                attnexp[:, 1024 * bi : 1024 * bi + 1024], ps[:],
                mybir.ActivationFunctionType.Exp, scale=SCALE,
            )

        paccs = {}

        def emit_proj_batch(n, bi):
            # kv chunk pair (bi, bi+16) — matches exp production order
            if n not in paccs:
                paccs[n] = pacc_ps.tile([65, 512], F32, tag="pacc",
                                        name=f"pacc{n}")
            pacc = paccs[n]
            for t in (bi, bi + 16):
                nc.tensor.matmul(
                    pacc[:], v_all[:, 65 * t : 65 * t + 65],
                    attnexp[:, aoff(t) : aoff(t) + 512],
                    start=(t == 0), stop=(t == 31),
                )

        def emit_finish_a(n):
            # free the PSUM accumulator ASAP: unnormalized proj rows (bf16,
            # feeds the out-projection) + raw denominator row
            pacc = paccs[n]
            projn_u = work.tile([64, 512], MDT, tag="projn", name=f"pn{n}")
            nc.vector.tensor_copy(projn_u[:], pacc[0:64, :])
            dn_sb = work.tile([65, 512], F32, tag="dn", name=f"dn{n}")
            nc.vector.tensor_copy(dn_sb[64:65, :], pacc[64:65, :])
            return projn_u, dn_sb

        def emit_finish_b(n, projn_u, dn_sb):
            # PE: broadcast raw denom + out-projection (nothing here waits on
            # a reciprocal). DVE then normalizes and applies bias + residual.
            qsl = slice(512 * n, 512 * n + 512)
            bc_ps = fin_ps.tile([64, 512], F32, tag="fin", name=f"bc{n}")
            nc.tensor.matmul(bc_ps[:], ones32[64:65, :], dn_sb[64:65, :],
                             start=True, stop=True)
            fps = fin_ps.tile([64, 512], F32, tag="fin", name=f"fps{n}")
            nc.tensor.matmul(fps[:], wo_sb[:], projn_u[:], start=True, stop=True)
            bc_sb = work.tile([64, 512], F32, tag="bc", name=f"bcs{n}")
            nc.vector.tensor_copy(bc_sb[:], bc_ps[:])
            with nc.allow_low_precision(reason="softmax denom reciprocal"):
                nc.vector.reciprocal(bc_sb[:], bc_sb[:])
            xb = work.tile([64, 512], F32, tag="xb", name=f"xb{n}")
            nc.vector.tensor_scalar_add(xb[:], x_sb[0:64, qsl], bo_sb[:])
            mn = work.tile([64, 512], F32, tag="mn", name=f"mn{n}")
            nc.vector.tensor_mul(mn[:], fps[:], bc_sb[:])
            nc.vector.tensor_add(out_sb[:, qsl], mn[:], xb[:])
            nc.sync.dma_start(out_d.ap()[:, qsl], out_sb[:, qsl])

        # ---- software-pipelined attention: 4 tiles x 16 pair-slots.
        # Per slot: scores-pair + exp(bi), attn@V of chunks 2(bi-2)..; the
        # previous tile's two spill batches and finish ride slots 0-1.
        # Tile 0's slots also carry the q/k slices and the v pairs. ----
        emit_qk_slice(0)
        V_SCHED = {0: [0, 1], 1: [2, 3], 2: [4], 3: [5], 4: [6, 7], 5: [8],
                   6: [9], 7: [10, 11], 8: [12], 9: [13], 10: [14], 11: [15]}
        QK_SCHED = {1: 1, 4: 2, 7: 3}

        pend = {}

        def emit_head(n, bi):
            if bi == 0:
                emit_proj_batch(n - 1, 14)
                emit_proj_batch(n - 1, 15)
            elif bi == 1:
                pend[n - 1] = emit_finish_a(n - 1)
            elif bi == 2:
                emit_finish_b(n - 1, *pend.pop(n - 1))

        for n in range(4):
            for bi in range(16):
                emit_scores_batch(n, bi)
                if n == 0:
                    if bi in QK_SCHED:
                        emit_qk_slice(QK_SCHED[bi])
                    for u in V_SCHED.get(bi, []):
                        emit_v_pair(u)
                else:
                    emit_head(n, bi)
                if bi >= 2:
                    emit_proj_batch(n, bi - 2)
        for bi in range(7):
            emit_head(4, bi)

    nc.compile()
    return nc


def host_prep(x, gamma, beta, Wq, bq, Wk, bk, Wv, bv, Wo, bo):
    """Build the 8 per-core input dicts."""
    f32 = lambda a: np.ascontiguousarray(np.asarray(a, np.float32))
    x = f32(x)
    gamma, beta = f32(gamma), f32(beta)
    Wq, Wk, Wv, Wo = f32(Wq), f32(Wk), f32(Wv), f32(Wo)
    bq, bk, bv, bo = f32(bq), f32(bk), f32(bv), f32(bo)

    wq_dup = np.ascontiguousarray(np.concatenate([Wq, Wq], axis=1))
    z = np.zeros((64, 64), np.float32)
    wk_blk = np.ascontiguousarray(np.block([[Wk, z], [z, Wk]]))
    wv_blk = np.ascontiguousarray(np.block([[Wv, z], [z, Wv]]))
    comb = np.zeros((128, 128), np.float32)
    comb[:64, :64] = 1.0 / 64.0
    comb[64:, 64:] = 1.0 / 64.0
    mdt_np = mybir.dt.np(MDT)
    m = lambda a: np.ascontiguousarray(a).astype(mdt_np)
    shared = {
        "wq": m(wq_dup), "wk": m(wk_blk), "wv": m(wv_blk), "wo": m(Wo),
        "bq": m(np.tile(bq, 2)[None]),
        "bk": m(np.tile(bk, 2)[None]),
        "bv": m(np.tile(bv, 2)[None]),
        "bo": np.ascontiguousarray(bo[:, None]),
        "gam": np.ascontiguousarray(np.tile(gamma, 2)[:, None]),
        "bet": np.ascontiguousarray(np.tile(beta, 2)[:, None]),
        "comb": comb,
    }
    in_maps = []
    for core in range(8):
        b, h = core // 2, core % 2
        xT = x[b].reshape(HW, C).T  # [64, 4096]
        halves = xT.reshape(C, 2, HALF)[:, [h, 1 - h], :]
        xp = np.ascontiguousarray(halves.transpose(1, 0, 2).reshape(128, HALF))
        in_maps.append({"xp": xp, **shared})
    return in_maps


def assemble(results, dtype):
    out = np.empty((B, HW, C), np.float32)
    for core in range(8):
        b, h = core // 2, core % 2
        out[b, HALF * h : HALF * h + HALF] = results[core]["out"].T
    return out.reshape(B, H, W, C).astype(dtype, copy=False)


_NC_CACHE = []


def kernel(x, gamma, beta, Wq, bq, Wk, bk, Wv, bv, Wo, bo):
    from concourse.bass_utils import run_bass_kernel_spmd

    if not _NC_CACHE:
        _NC_CACHE.append(build_nc())
    nc = _NC_CACHE[0]
    in_maps = host_prep(x, gamma, beta, Wq, bq, Wk, bk, Wv, bv, Wo, bo)
    res = run_bass_kernel_spmd(nc, in_maps, core_ids=list(range(8)))
    return assemble(res.results, np.asarray(x).dtype)


if __name__ == "__main__":
    rng = np.random.default_rng(0)
    inputs = {
        "x": rng.standard_normal((B, H, W, C)).astype(np.float32),
        "gamma": np.ones(C, np.float32), "beta": np.zeros(C, np.float32),
        "Wq": (rng.standard_normal((C, C)) / 8).astype(np.float32),
        "bq": np.zeros(C, np.float32),
        "Wk": (rng.standard_normal((C, C)) / 8).astype(np.float32),
        "bk": np.zeros(C, np.float32),
        "Wv": (rng.standard_normal((C, C)) / 8).astype(np.float32),
        "bv": np.zeros(C, np.float32),
        "Wo": (rng.standard_normal((C, C)) / 8).astype(np.float32),
        "bo": np.zeros(C, np.float32),
    }
    out = kernel(**inputs)
    print("kernel ran, out shape", out.shape, out.dtype)
